# revision 1
# baseline (speedup 1.0000x reference)
"""ParticleFlowNetwork TRN2 Bass kernel.

Network (B=4096, P=128, IN=4, H=100):
    h = x @ W0 + b0            (no ReLU)
    h = relu(h @ W1 + b1)
    h = relu(h @ W2 + b2)
    h = h @ W3 + b3
    lat = sum over P          -> [B, 8]
    z = relu(lat @ U0 + c0); z = relu(z @ U1 + c1); z = relu(z @ U2 + c2)
    z = relu(z @ U3 + c3); out = softmax(z @ U4 + c4)

Algebraic folds applied on host (all linear, exact in fp32):
  * W01 = W0 @ W1, b1' = b0 @ W1 + b1      (no ReLU after layer 0)
  * pooling moved before W3 (linear), and W3U0 = W3 @ U0,
    c0' = P * (b3 @ U0) + c0               (lat never materialized)
  * 2-class softmax == sigmoid of logit difference:
    u4d = [U4[:,0]-U4[:,1], U4[:,1]-U4[:,0]], c4d likewise.

Device layout: pure data parallel, batch 4096 -> 8 cores x 512 rows.
Per core everything is kept "transposed" (hidden on partitions, tokens on
the free dim). x is transposed on-chip with a regular matmul
(x_tile as stationary operand, identity as moving operand).
Matmul operands are bf16 (fp32 matmul costs 4 cycles/col, bf16 1);
PSUM accumulation is fp32.  Measured output rel-err vs fp32 ref ~1.4e-4.
"""

import os
from contextlib import ExitStack

import numpy as np
import ml_dtypes

import concourse.bass as bass
import concourse.tile as tile
from concourse import bacc, mybir
from concourse._compat import with_exitstack
from concourse.bass_utils import run_bass_kernel_spmd

F32 = mybir.dt.float32
BF16 = mybir.dt.bfloat16
AF = mybir.ActivationFunctionType
ALU = mybir.AluOpType

B, P, IN, H = 4096, 128, 4, 100
NCORES = 8
BS = B // NCORES          # 512 batch rows per core
SUP = 4                   # supertiles per core
SB = BS // SUP            # 128 batch rows per supertile
ROW = P * IN              # 512 floats per batch row
NT = P // 4               # 32 particle-tiles per supertile (4 particles each)


@with_exitstack
def _body(ctx: ExitStack, tc: "tile.TileContext", d):
    nc = tc.nc

    const = ctx.enter_context(tc.tile_pool(name="const", bufs=1))
    xf_pool = ctx.enter_context(tc.tile_pool(name="xf", bufs=2))
    xb_pool = ctx.enter_context(tc.tile_pool(name="xb", bufs=2))
    xt_pool = ctx.enter_context(tc.tile_pool(name="xt", bufs=2))
    h_pool = ctx.enter_context(tc.tile_pool(name="h", bufs=3))
    z_pool = ctx.enter_context(tc.tile_pool(name="z", bufs=1))
    zr_pool = ctx.enter_context(tc.tile_pool(name="zr", bufs=2))
    ps_xt = ctx.enter_context(tc.tile_pool(name="ps_xt", bufs=1, space="PSUM"))
    ps_h1 = ctx.enter_context(tc.tile_pool(name="ps_h1", bufs=2, space="PSUM"))
    ps_h2 = ctx.enter_context(tc.tile_pool(name="ps_h2", bufs=2, space="PSUM"))
    ps_z = ctx.enter_context(tc.tile_pool(name="ps_z", bufs=2, space="PSUM"))

    # --- constants ---
    w01 = const.tile([128, NT * H], BF16)
    nc.sync.dma_start(w01[:], d["w01"].ap())
    w2 = const.tile([H, H], BF16)
    nc.sync.dma_start(w2[:], d["w2"].ap())
    w3u0 = const.tile([H, H], BF16)
    nc.sync.dma_start(w3u0[:], d["w3u0"].ap())
    u1 = const.tile([H, H], BF16)
    nc.sync.dma_start(u1[:], d["u1"].ap())
    u2 = const.tile([H, H], BF16)
    nc.sync.dma_start(u2[:], d["u2"].ap())
    u3 = const.tile([H, H], BF16)
    nc.sync.dma_start(u3[:], d["u3"].ap())
    u4d = const.tile([H, 2], BF16)
    nc.sync.dma_start(u4d[:], d["u4d"].ap())
    ident = const.tile([128, 128], BF16)
    nc.sync.dma_start(ident[:], d["ident"].ap())
    bias = const.tile([128, 8], F32)
    nc.sync.dma_start(bias[:], d["bias"].ap())
    b1p = bias[:H, 0:1]
    b2 = bias[:H, 1:2]
    c0p = bias[:H, 2:3]
    c1 = bias[:H, 3:4]
    c2 = bias[:H, 4:5]
    c3 = bias[:H, 5:6]
    c4d = bias[0:2, 6:7]

    z0_all = z_pool.tile([H, BS], BF16)  # relu(pooled @ W3U0 + c0'), all 512 rows

    for s in range(SUP):
        # load 128 batch rows (one per partition, 512 contiguous floats each)
        x_f = xf_pool.tile([128, ROW], F32, tag="xf")
        nc.sync.dma_start(x_f[:], d["x"].ap()[s * SB:(s + 1) * SB, :])
        x_b = xb_pool.tile([128, ROW], BF16, tag="xb")
        nc.vector.tensor_copy(x_b[:], x_f[:])

        # transpose via matmul: out = x_slice.T @ I  -> [row-elem, batch]
        xt_ps = ps_xt.tile([128, ROW], F32, tag="xt_ps")
        for m in range(4):
            nc.tensor.matmul(
                xt_ps[:, 128 * m:128 * (m + 1)],
                x_b[:, 128 * m:128 * (m + 1)],
                ident[:],
                start=True, stop=True,
            )
        xt_sb = xt_pool.tile([128, ROW], BF16, tag="xt_sb")
        nc.scalar.copy(xt_sb[:], xt_ps[:])

        # z0 accumulator for this supertile's 128 batch rows (4 col-blocks)
        z0_ps = ps_z.tile([H, ROW], F32, tag="z0_ps")

        for t in range(NT):
            h1_ps = ps_h1.tile([H, ROW], F32, tag="h1_ps")
            nc.tensor.matmul(
                h1_ps[:], w01[:, H * t:H * (t + 1)], xt_sb[:],
                start=True, stop=True,
            )
            h1_sb = h_pool.tile([H, ROW], BF16, tag="h1_sb")
            if t % 2 == 0:
                nc.scalar.activation(h1_sb[:], h1_ps[:], AF.Relu, bias=b1p)
            else:
                nc.vector.tensor_scalar(
                    h1_sb[:], h1_ps[:], b1p, 0.0, ALU.add, ALU.max)

            h2_ps = ps_h2.tile([H, ROW], F32, tag="h2_ps")
            nc.tensor.matmul(h2_ps[:], w2[:], h1_sb[:], start=True, stop=True)
            h2_sb = h_pool.tile([H, ROW], BF16, tag="h2_sb")
            if t % 2 == 0:
                nc.vector.tensor_scalar(
                    h2_sb[:], h2_ps[:], b2, 0.0, ALU.add, ALU.max)
            else:
                nc.scalar.activation(h2_sb[:], h2_ps[:], AF.Relu, bias=b2)

            # accumulate pooled @ W3U0 over all particles of the supertile
            nc.tensor.matmul(
                z0_ps[:], w3u0[:], h2_sb[:],
                start=(t == 0), stop=(t == NT - 1),
            )

        # fold the 4 col-blocks (partial particle sums) and finish z0
        z0_raw = zr_pool.tile([H, ROW], F32, tag="z0_raw")
        nc.vector.tensor_copy(z0_raw[:], z0_ps[:])
        zf0 = zr_pool.tile([H, SB], F32, tag="zf0")
        nc.vector.tensor_tensor(
            zf0[:], z0_raw[:, 0:SB], z0_raw[:, SB:2 * SB], ALU.add)
        zf1 = zr_pool.tile([H, SB], F32, tag="zf1")
        nc.vector.tensor_tensor(
            zf1[:], z0_raw[:, 2 * SB:3 * SB], z0_raw[:, 3 * SB:4 * SB], ALU.add)
        zf = zr_pool.tile([H, SB], F32, tag="zf")
        nc.vector.tensor_tensor(zf[:], zf0[:], zf1[:], ALU.add)
        nc.scalar.activation(
            z0_all[:, s * SB:(s + 1) * SB], zf[:], AF.Relu, bias=c0p)

    # --- event head on all 512 batch rows ---
    zin = z0_all
    for u, c in ((u1, c1), (u2, c2), (u3, c3)):
        zh_ps = ps_h1.tile([H, BS], F32, tag="h1_ps")
        nc.tensor.matmul(zh_ps[:], u[:], zin[:], start=True, stop=True)
        zh = h_pool.tile([H, BS], BF16, tag="h1_sb")
        nc.scalar.activation(zh[:], zh_ps[:], AF.Relu, bias=c)
        zin = zh

    d_ps = ps_h2.tile([2, BS], F32, tag="h2_ps")
    nc.tensor.matmul(d_ps[:], u4d[:], zin[:], start=True, stop=True)
    probs = h_pool.tile([2, BS], F32, tag="probs")
    nc.scalar.activation(probs[:], d_ps[:], AF.Sigmoid, bias=c4d)
    nc.sync.dma_start(d["y"].ap(), probs[:])


def _build():
    nc = bacc.Bacc("TRN2", target_bir_lowering=False, debug=False)
    d = {}
    d["x"] = nc.dram_tensor("x", [BS, ROW], F32, kind="ExternalInput")
    d["w01"] = nc.dram_tensor("w01", [128, NT * H], BF16, kind="ExternalInput")
    d["w2"] = nc.dram_tensor("w2", [H, H], BF16, kind="ExternalInput")
    d["w3u0"] = nc.dram_tensor("w3u0", [H, H], BF16, kind="ExternalInput")
    d["u1"] = nc.dram_tensor("u1", [H, H], BF16, kind="ExternalInput")
    d["u2"] = nc.dram_tensor("u2", [H, H], BF16, kind="ExternalInput")
    d["u3"] = nc.dram_tensor("u3", [H, H], BF16, kind="ExternalInput")
    d["u4d"] = nc.dram_tensor("u4d", [H, 2], BF16, kind="ExternalInput")
    d["ident"] = nc.dram_tensor("ident", [128, 128], BF16, kind="ExternalInput")
    d["bias"] = nc.dram_tensor("bias", [128, 8], F32, kind="ExternalInput")
    d["y"] = nc.dram_tensor("y", [2, BS], F32, kind="ExternalOutput")

    with tile.TileContext(nc) as tc:
        _body(tc, d)
    nc.compile()
    return nc


_NC = None


def _get_nc():
    global _NC
    if _NC is None:
        _NC = _build()
    return _NC


def _prep_inputs(inputs):
    f32 = np.float32
    bf16 = ml_dtypes.bfloat16
    W0, b0 = np.asarray(inputs["W0"], f32), np.asarray(inputs["b0"], f32)
    W1, b1 = np.asarray(inputs["W1"], f32), np.asarray(inputs["b1"], f32)
    W2, b2 = np.asarray(inputs["W2"], f32), np.asarray(inputs["b2"], f32)
    W3, b3 = np.asarray(inputs["W3"], f32), np.asarray(inputs["b3"], f32)
    U0, c0 = np.asarray(inputs["U0"], f32), np.asarray(inputs["c0"], f32)
    U1, c1 = np.asarray(inputs["U1"], f32), np.asarray(inputs["c1"], f32)
    U2, c2 = np.asarray(inputs["U2"], f32), np.asarray(inputs["c2"], f32)
    U3, c3 = np.asarray(inputs["U3"], f32), np.asarray(inputs["c3"], f32)
    U4, c4 = np.asarray(inputs["U4"], f32), np.asarray(inputs["c4"], f32)

    W01 = W0 @ W1                                # [4, H]
    b1p = b0 @ W1 + b1                           # [H]
    W3U0 = W3 @ U0                               # [H, H]
    c0p = np.float32(P) * (b3 @ U0) + c0         # [H]
    u4diff = U4[:, 0] - U4[:, 1]                 # [H]
    u4d = np.stack([u4diff, -u4diff], axis=1)    # [H, 2]
    c4d = np.array([c4[0] - c4[1], c4[1] - c4[0]], f32)

    # padded W01: slice t holds W01 rows at partitions 4t..4t+3
    w01p = np.zeros((128, NT * H), f32)
    for t in range(NT):
        w01p[4 * t:4 * t + 4, H * t:H * (t + 1)] = W01

    bias = np.zeros((128, 8), f32)
    bias[:H, 0] = b1p
    bias[:H, 1] = b2
    bias[:H, 2] = c0p
    bias[:H, 3] = c1
    bias[:H, 4] = c2
    bias[:H, 5] = c3
    bias[0:2, 6] = c4d

    shared = {
        "w01": w01p.astype(bf16),
        "w2": W2.astype(bf16),
        "w3u0": W3U0.astype(bf16),
        "u1": U1.astype(bf16),
        "u2": U2.astype(bf16),
        "u3": U3.astype(bf16),
        "u4d": u4d.astype(bf16),
        "ident": np.eye(128, dtype=f32).astype(bf16),
        "bias": bias,
    }
    x = np.ascontiguousarray(np.asarray(inputs["x"], f32).reshape(B, ROW))
    in_maps = []
    for k in range(NCORES):
        m = dict(shared)
        m["x"] = np.ascontiguousarray(x[k * BS:(k + 1) * BS])
        in_maps.append(m)
    return in_maps


def kernel(**inputs):
    nc = _get_nc()
    in_maps = _prep_inputs(inputs)
    res = run_bass_kernel_spmd(nc, in_maps, list(range(NCORES)))
    out = np.empty((B, 2), np.float32)
    for k in range(NCORES):
        y = np.asarray(res.results[k]["y"])  # [2, BS]
        out[k * BS:(k + 1) * BS, 0] = y[0]
        out[k * BS:(k + 1) * BS, 1] = y[1]
    return out
